# revision 32
# baseline (speedup 1.0000x reference)
"""HRA (Householder Reflection Adaptation) forward kernel for Trainium2.

Math: out = x @ Q with Q = prod_i (I - 2 u_i u_i^T), u_i = normalized columns
of hra_u [4096, 8].  Using the compact WY representation:
    Q = I - U T U^T      (T upper-triangular 8x8, diag=2)
    out = x - (x @ A) @ U^T,   A = U @ T
so the device only does two skinny matmuls per tile plus a subtract.

Sharding: data-parallel over rows. x [4,2048,4096] -> [8192, 4096]; each of
8 cores gets 1024 contiguous rows. A and U^T are tiny and replicated.

Per-core pipeline (256-row blocks, 4 per core, software-pipelined):

  All DMA rides ONE sync HWDGE ring: input prefetches are enqueued first, so
  the FIFO gives a clean read phase at full HBM rate (~430 GB/s), then the
  write phase streams the outputs, instead of slow mixed read/write traffic.

  front(b): per 2-chunk group: 4 PE transposes -> PSUM strip, ACT copy ->
    SBUF x^T (rounded to f32r), then the accumulating f32r proj matmul for
    the group, emitted TWO groups late so it never stalls the PE FIFO on
    the ACT copy.  A is replicated into 2 column groups on device (one
    compact DMA + 2 DVE copies), so P^T lands in PSUM already replicated at
    partitions {0-7, 32-39}.

  back(b-1), interleaved into front(b): the replicated P^T strips let the
    K=8 update matmuls run pairwise via PE row-group packing: 2 concurrent
    N=512 f32r matmuls into one 2-bank PSUM tile, one [128,1024] DVE
    subtract in place, then the out piece queues on the sync ring behind
    the remaining reads.  One pso tile per pair with bufs=2 leaves a full
    pair of slack so the PE FIFO never blocks on DVE.

  A warm-up burst on the identity (the first DMA to land) opens the PE HAM
  clock-gate before the first real transposes.  Transposes stay plain f32:
  the f32 transpose lowers to a HAM-counted matmul, while the f32r
  transpose-mode variant does not count as PE activity and lets the clock
  gate throttle the whole kernel to 1.2 GHz.
"""

import os
import sys

for _p in ("/opt/trn_rl_repo", "/root/.axon_site", "/root/.axon_site/_ro/trn_rl_repo",
           "/root/.axon_site/_ro/pypackages"):
    if os.path.isdir(_p) and _p not in sys.path:
        sys.path.append(_p)

import numpy as np

import concourse.bass as bass
import concourse.mybir as mybir
import concourse.tile as tile
from concourse import bacc
from concourse.bass_utils import run_bass_kernel_spmd

B, S, D, R = 4, 2048, 4096, 8
N_CORES = 8
ROWS = B * S                      # 8192
ROWS_PER_CORE = ROWS // N_CORES   # 1024
P = 128
D_CHUNKS = D // P                 # 32
UPD_CHUNKS = D // 512             # 8
QUAD = 40                         # replicated-proj partition span: 32 + 8

F32 = mybir.dt.float32
F32R = mybir.dt.float32r
BF16 = mybir.dt.bfloat16

_CACHE = {}


def _householder_wy(hra_u: np.ndarray):
    """Return (A, UT) f32 with out = x - (x @ A) @ UT."""
    u = hra_u.astype(np.float32)
    u = u / np.linalg.norm(u, axis=0, keepdims=True)
    U = u.astype(np.float64)
    T = np.zeros((R, R), np.float64)
    for k in range(R):
        T[k, k] = 2.0
        if k:
            T[:k, k] = -2.0 * (T[:k, :k] @ (U[:, :k].T @ U[:, k]))
    A = (U @ T).astype(np.float32)          # [D, R]
    return A, np.ascontiguousarray(u.T)     # [R, D]


J = 2                             # 128-row tiles per block
BLK = J * P                       # 256 rows per block
N_BLKS = ROWS_PER_CORE // BLK     # 4 blocks per core
LAG = 2                           # groups the proj matmul trails transposes


def _build_program():
    nc = bacc.Bacc(trn_type="TRN2")
    x = nc.dram_tensor("x", (ROWS_PER_CORE, D), F32, kind="ExternalInput")
    a = nc.dram_tensor("a", (P, D_CHUNKS * R), F32R, kind="ExternalInput")
    ut = nc.dram_tensor("ut", (R, D), F32R, kind="ExternalInput")
    ident = nc.dram_tensor("ident", (P, P), F32, kind="ExternalInput")
    out = nc.dram_tensor("out", (ROWS_PER_CORE, D), F32, kind="ExternalOutput")

    xd = x.rearrange("(b j p) d -> b p j d", p=P, j=J)
    od = out.rearrange("(b j p) d -> b p j d", p=P, j=J)

    with tile.TileContext(nc) as tc:
        with (
            tc.tile_pool(name="const", bufs=1) as const,
            tc.tile_pool(name="xp", bufs=4) as x_pool,
            tc.tile_pool(name="xtp", bufs=4) as xt_pool,
            tc.tile_pool(name="ptp", bufs=2) as pt_pool,
            tc.tile_pool(name="pst", bufs=3, space="PSUM") as pst_pool,
            tc.tile_pool(name="psp", bufs=1, space="PSUM") as psp_pool,
            tc.tile_pool(name="pso", bufs=2, space="PSUM") as pso_pool,
        ):
            # DMA order on the sync ring: ident (warm-up), compact a, block-0
            # first halves (first transposes), ut replicas, block-0 second
            # halves, blocks 1-3 (one 4MB DMA each -- fewer issues means the
            # 8 DMAHW completion lanes never starve the ring).
            ident_sb = const.tile([P, P], F32)
            nc.sync.dma_start(ident_sb, ident[:, :])
            # load a compactly (one contiguous DMA; tiny strided DMAs would
            # be descriptor-bound), then DVE builds the column replica
            a_c = const.tile([P, D_CHUNKS, R], F32R)
            nc.sync.dma_start(a_c, a.rearrange("p (c r) -> p c r", r=R))
            # x transfers stay whole per-j tiles: 16KB contiguous per
            # partition keeps HBM reads at line rate (column-half splits
            # read 8KB-strided runs and drop to ~60% bandwidth)
            xbs = []
            xb0 = x_pool.tile([P, J, D], F32, tag="xb")
            xbs.append(xb0)
            nc.sync.dma_start(xb0[:, 0], xd[0, :, 0])
            ut_sb = const.tile([QUAD, D], F32R)
            for g in range(2):
                nc.sync.dma_start(ut_sb[32 * g:32 * g + R, :], ut[:, :])
            nc.sync.dma_start(xb0[:, 1], xd[0, :, 1])
            for b in range(1, N_BLKS):
                xb = x_pool.tile([P, J, D], F32, tag="xb")
                xbs.append(xb)
                for j in range(J):
                    nc.sync.dma_start(xb[:, j], xd[b, :, j])

            a_sb = const.tile([P, D_CHUNKS, QUAD], F32R)
            for g in range(2):
                nc.vector.tensor_copy(a_sb[:, :, 32 * g:32 * g + R], a_c)

            # warm-up burst on ident only (lands first): dense f32 matmuls
            # open the PE HAM clock-gate before the first real transposes
            warm_t = pst_pool.tile([P, 2, BLK], F32, tag="ps_t")
            nc.tensor.transpose(warm_t[:, 0, :P], ident_sb, ident_sb)
            warm = pso_pool.tile([P, 2, 512], F32, tag="ps_o")
            for _ in range(26):
                nc.tensor.matmul(warm[:, 0, :P], ident_sb, ident_sb,
                                 start=True, stop=True)

            def back_units(b, pt):
                """yield per-pair units for block b's update phase: each unit
                is 2 row-group-packed N=512 f32r matmuls into one 2-bank
                PSUM tile, then a [128,1024] DVE subtract + sync-ring DMA."""
                xb = xbs[b]
                combos = [(j, c) for j in range(J) for c in range(UPD_CHUNKS)]

                def pair(s):
                    ps_o = pso_pool.tile([P, 2, 512], F32, tag="ps_o",
                                         name="ps_o")
                    for g in range(2):
                        j, c = combos[2 * s + g]
                        nc.tensor.matmul(
                            ps_o[:, g, :],
                            pt[32 * g:32 * g + R, j * P:(j + 1) * P],
                            ut_sb[32 * g:32 * g + R, c * 512:(c + 1) * 512],
                            start=True,
                            stop=True,
                            tile_position=(32 * g, 0),
                        )
                    j, c0 = combos[2 * s]
                    dst = xb[:, j, c0 * 512:(c0 + 2) * 512]
                    nc.vector.tensor_sub(dst, dst, ps_o)
                    # the write queues on the sync ring behind all reads
                    nc.sync.dma_start(od[b, :, j, c0 * 512:(c0 + 2) * 512],
                                      dst)

                for s in range(len(combos) // 2):
                    yield lambda s=s: pair(s)

            n_g = D_CHUNKS // 2
            ps_ps = {}
            xts = {}

            def transposes(b, g):
                ps_t = pst_pool.tile([P, 2, BLK], F32, tag="ps_t")
                for i in range(2):
                    k = 2 * g + i
                    for j in range(J):
                        nc.tensor.transpose(
                            ps_t[:, i, j * P:(j + 1) * P],
                            xbs[b][:, j, k * P:(k + 1) * P],
                            ident_sb,
                        )
                xt_g = xt_pool.tile([P, 2, BLK], F32R, tag="xt_g")
                nc.scalar.copy(xt_g, ps_t)
                xts[(b, g)] = xt_g

            def proj(b, g):
                if g == 0:
                    ps_ps[b] = psp_pool.tile([QUAD, BLK], F32, tag="ps_p",
                                             name="ps_p")
                for i in range(2):
                    k = 2 * g + i
                    nc.tensor.matmul(
                        ps_ps[b],
                        a_sb[:, k],
                        xts[(b, g)][:, i],
                        start=(k == 0),
                        stop=(k == D_CHUNKS - 1),
                    )
                if g == n_g - 1:
                    del xts[(b, g)]
                    pt = pt_pool.tile([QUAD, BLK], F32R, tag="pt")
                    nc.vector.tensor_copy(pt, ps_ps[b])
                    pts[b] = pt
                    pending.extend(back_units(b, pt))
                else:
                    del xts[(b, g)]

            pts = {}
            pending = []
            n_tot = N_BLKS * n_g
            # merged 2-group slots: [t x8][p x4][pair] -- fewer instruction-
            # type transitions on the PE stream (each t->p / p->pair boundary
            # exposes ~90-200ns of weight-load latency that chains hide).
            # tile_wait_until pins the scheduler's dispatch order to the slot
            # sequence (its internal cost model otherwise reorders the next
            # block's transposes behind a whole update phase)
            G = 0
            w = 0
            for G in range(0, n_tot + LAG, 2):
                w += 1
                with tc.tile_wait_until(w):
                    for d in range(2):
                        Gt = G + d
                        if Gt < n_tot:
                            transposes(Gt // n_g, Gt % n_g)
                    # two back pairs every other slot, placed between the
                    # transpose chain and the projs: pair matmuls back-to-back
                    # amortize entry/exit weight-load exposure, and ending the
                    # slot on projs makes the next slot's transpose LDWs hide
                    # under the proj streams (pair->transpose costs ~322ns,
                    # proj->transpose is free)
                    if (G // 2) % 2 == 0:
                        for _ in range(2):
                            if pending:
                                pending.pop(0)()
                    for d in range(2):
                        Gp = G + d - LAG
                        if 0 <= Gp < n_tot:
                            proj(Gp // n_g, Gp % n_g)
            while pending:
                w += 1
                with tc.tile_wait_until(w):
                    pending.pop(0)()

    nc.compile()
    return nc


def _get_program():
    if "nc" not in _CACHE:
        _CACHE["nc"] = _build_program()
    return _CACHE["nc"]


def kernel(input, hra_u, **run_kwargs):
    input = np.ascontiguousarray(np.asarray(input, dtype=np.float32))
    hra_u = np.asarray(hra_u, dtype=np.float32)

    A, UT = _householder_wy(hra_u)
    # pack A compactly: partition p holds A[c*128+p, :] at free offset c*R;
    # the device replicates it into 2 column groups with DVE copies
    a_packed = np.ascontiguousarray(
        A.reshape(D_CHUNKS, P, R).transpose(1, 0, 2).reshape(P, D_CHUNKS * R)
    )
    ident = np.eye(P, dtype=np.float32)

    x_flat = input.reshape(ROWS, D)
    in_maps = [
        {
            "x": x_flat[c * ROWS_PER_CORE:(c + 1) * ROWS_PER_CORE],
            "a": a_packed,
            "ut": UT,
            "ident": ident,
        }
        for c in range(N_CORES)
    ]

    nc = _get_program()
    res = run_bass_kernel_spmd(nc, in_maps, core_ids=list(range(N_CORES)),
                               **run_kwargs)
    out = np.concatenate([r["out"] for r in res.results], axis=0)
    if run_kwargs:
        kernel.last_results = res
    return out.reshape(B, S, D)


# revision 34
# speedup vs baseline: 1.0291x; 1.0291x over previous
"""HRA (Householder Reflection Adaptation) forward kernel for Trainium2.

Math: out = x @ Q with Q = prod_i (I - 2 u_i u_i^T), u_i = normalized columns
of hra_u [4096, 8].  Using the compact WY representation:
    Q = I - U T U^T      (T upper-triangular 8x8, diag=2)
    out = x - (x @ A) @ U^T,   A = U @ T
so the device only does two skinny matmuls per tile plus a subtract.

Sharding: data-parallel over rows. x [4,2048,4096] -> [8192, 4096]; each of
8 cores gets 1024 contiguous rows. A and U^T are tiny and replicated.

Per-core pipeline (256-row blocks, 4 per core, software-pipelined):

  All DMA rides ONE sync HWDGE ring: input prefetches are enqueued first, so
  the FIFO gives a clean read phase at full HBM rate (~430 GB/s), then the
  write phase streams the outputs, instead of slow mixed read/write traffic.

  front(b): per 2-chunk group: 4 PE transposes -> PSUM strip, ACT copy ->
    SBUF x^T (rounded to f32r), then the accumulating f32r proj matmul for
    the group, emitted TWO groups late so it never stalls the PE FIFO on
    the ACT copy.  A is replicated into 2 column groups on device (one
    compact DMA + 2 DVE copies), so P^T lands in PSUM already replicated at
    partitions {0-7, 32-39}.

  back(b-1), interleaved into front(b): the replicated P^T strips let the
    K=8 update matmuls run pairwise via PE row-group packing: 2 concurrent
    N=512 f32r matmuls into one 2-bank PSUM tile, one [128,1024] DVE
    subtract in place, then the out piece queues on the sync ring behind
    the remaining reads.  One pso tile per pair with bufs=2 leaves a full
    pair of slack so the PE FIFO never blocks on DVE.

  A warm-up burst on the identity (the first DMA to land) opens the PE HAM
  clock-gate before the first real transposes.  Transposes stay plain f32:
  the f32 transpose lowers to a HAM-counted matmul, while the f32r
  transpose-mode variant does not count as PE activity and lets the clock
  gate throttle the whole kernel to 1.2 GHz.
"""

import os
import sys

for _p in ("/opt/trn_rl_repo", "/root/.axon_site", "/root/.axon_site/_ro/trn_rl_repo",
           "/root/.axon_site/_ro/pypackages"):
    if os.path.isdir(_p) and _p not in sys.path:
        sys.path.append(_p)

import numpy as np

import concourse.bass as bass
import concourse.mybir as mybir
import concourse.tile as tile
from concourse import bacc
from concourse.bass_utils import run_bass_kernel_spmd

B, S, D, R = 4, 2048, 4096, 8
N_CORES = 8
ROWS = B * S                      # 8192
ROWS_PER_CORE = ROWS // N_CORES   # 1024
P = 128
D_CHUNKS = D // P                 # 32
UPD_CHUNKS = D // 512             # 8
QUAD = 40                         # replicated-proj partition span: 32 + 8

F32 = mybir.dt.float32
F32R = mybir.dt.float32r
BF16 = mybir.dt.bfloat16

_CACHE = {}


def _householder_wy(hra_u: np.ndarray):
    """Return (A, UT) f32 with out = x - (x @ A) @ UT."""
    u = hra_u.astype(np.float32)
    u = u / np.linalg.norm(u, axis=0, keepdims=True)
    U = u.astype(np.float64)
    T = np.zeros((R, R), np.float64)
    for k in range(R):
        T[k, k] = 2.0
        if k:
            T[:k, k] = -2.0 * (T[:k, :k] @ (U[:, :k].T @ U[:, k]))
    A = (U @ T).astype(np.float32)          # [D, R]
    return A, np.ascontiguousarray(u.T)     # [R, D]


J = 2                             # 128-row tiles per block
BLK = J * P                       # 256 rows per block
N_BLKS = ROWS_PER_CORE // BLK     # 4 blocks per core
LAG = 2                           # groups the proj matmul trails transposes


def _build_program():
    nc = bacc.Bacc(trn_type="TRN2")
    x = nc.dram_tensor("x", (ROWS_PER_CORE, D), F32, kind="ExternalInput")
    a = nc.dram_tensor("a", (P, D_CHUNKS * R), F32R, kind="ExternalInput")
    ut = nc.dram_tensor("ut", (R, D), F32R, kind="ExternalInput")
    ident = nc.dram_tensor("ident", (P, P), F32, kind="ExternalInput")
    out = nc.dram_tensor("out", (ROWS_PER_CORE, D), F32, kind="ExternalOutput")

    xd = x.rearrange("(b j p) d -> b p j d", p=P, j=J)
    od = out.rearrange("(b j p) d -> b p j d", p=P, j=J)

    with tile.TileContext(nc) as tc:
        with (
            tc.tile_pool(name="const", bufs=1) as const,
            tc.tile_pool(name="xp", bufs=4) as x_pool,
            tc.tile_pool(name="xtp", bufs=4) as xt_pool,
            tc.tile_pool(name="ptp", bufs=2) as pt_pool,
            tc.tile_pool(name="pst", bufs=3, space="PSUM") as pst_pool,
            tc.tile_pool(name="psp", bufs=1, space="PSUM") as psp_pool,
            tc.tile_pool(name="pso", bufs=2, space="PSUM") as pso_pool,
        ):
            # DMA order on the sync ring: ident (warm-up), compact a, block-0
            # first halves (first transposes), ut replicas, block-0 second
            # halves, blocks 1-3 (one 4MB DMA each -- fewer issues means the
            # 8 DMAHW completion lanes never starve the ring).
            ident_sb = const.tile([P, P], F32)
            nc.sync.dma_start(ident_sb, ident[:, :])
            # load a compactly (one contiguous DMA; tiny strided DMAs would
            # be descriptor-bound), then DVE builds the column replica
            a_c = const.tile([P, D_CHUNKS, R], F32R)
            nc.sync.dma_start(a_c, a.rearrange("p (c r) -> p c r", r=R))
            # x transfers stay whole per-j tiles: 16KB contiguous per
            # partition keeps HBM reads at line rate (column-half splits
            # read 8KB-strided runs and drop to ~60% bandwidth)
            xbs = []
            xb0 = x_pool.tile([P, J, D], F32, tag="xb")
            xbs.append(xb0)
            nc.sync.dma_start(xb0[:, 0], xd[0, :, 0])
            ut_sb = const.tile([QUAD, D], F32R)
            for g in range(2):
                nc.sync.dma_start(ut_sb[32 * g:32 * g + R, :], ut[:, :])
            nc.sync.dma_start(xb0[:, 1], xd[0, :, 1])
            for b in range(1, N_BLKS):
                xb = x_pool.tile([P, J, D], F32, tag="xb")
                xbs.append(xb)
                for j in range(J):
                    nc.sync.dma_start(xb[:, j], xd[b, :, j])

            a_sb = const.tile([P, D_CHUNKS, QUAD], F32R)
            for g in range(2):
                nc.vector.tensor_copy(a_sb[:, :, 32 * g:32 * g + R], a_c)

            # warm-up burst on ident only (lands first): dense f32 matmuls
            # open the PE HAM clock-gate before the first real transposes
            warm_t = pst_pool.tile([P, 2, BLK], F32, tag="ps_t")
            nc.tensor.transpose(warm_t[:, 0, :P], ident_sb, ident_sb)
            warm = pso_pool.tile([P, 2, 512], F32, tag="ps_o")
            for _ in range(40):
                nc.tensor.matmul(warm[:, 0, :P], ident_sb, ident_sb,
                                 start=True, stop=True)

            def back_units(b, pt):
                """yield per-pair units for block b's update phase: each unit
                is 2 row-group-packed N=512 f32r matmuls into one 2-bank
                PSUM tile, then a [128,1024] DVE subtract + sync-ring DMA."""
                xb = xbs[b]
                combos = [(j, c) for j in range(J) for c in range(UPD_CHUNKS)]

                def pair(s):
                    ps_o = pso_pool.tile([P, 2, 512], F32, tag="ps_o",
                                         name="ps_o")
                    for g in range(2):
                        j, c = combos[2 * s + g]
                        nc.tensor.matmul(
                            ps_o[:, g, :],
                            pt[32 * g:32 * g + R, j * P:(j + 1) * P],
                            ut_sb[32 * g:32 * g + R, c * 512:(c + 1) * 512],
                            start=True,
                            stop=True,
                            tile_position=(32 * g, 0),
                        )
                    j, c0 = combos[2 * s]
                    dst = xb[:, j, c0 * 512:(c0 + 2) * 512]
                    nc.vector.tensor_sub(dst, dst, ps_o)
                    # the write queues on the sync ring behind all reads
                    nc.sync.dma_start(od[b, :, j, c0 * 512:(c0 + 2) * 512],
                                      dst)

                for s in range(len(combos) // 2):
                    yield lambda s=s: pair(s)

            n_g = D_CHUNKS // 2
            ps_ps = {}
            xts = {}

            def transposes(b, g):
                ps_t = pst_pool.tile([P, 2, BLK], F32, tag="ps_t")
                for i in range(2):
                    k = 2 * g + i
                    for j in range(J):
                        nc.tensor.transpose(
                            ps_t[:, i, j * P:(j + 1) * P],
                            xbs[b][:, j, k * P:(k + 1) * P],
                            ident_sb,
                        )
                xt_g = xt_pool.tile([P, 2, BLK], F32R, tag="xt_g")
                nc.scalar.copy(xt_g, ps_t)
                xts[(b, g)] = xt_g

            def proj(b, g):
                if g == 0:
                    ps_ps[b] = psp_pool.tile([QUAD, BLK], F32, tag="ps_p",
                                             name="ps_p")
                for i in range(2):
                    k = 2 * g + i
                    nc.tensor.matmul(
                        ps_ps[b],
                        a_sb[:, k],
                        xts[(b, g)][:, i],
                        start=(k == 0),
                        stop=(k == D_CHUNKS - 1),
                    )
                if g == n_g - 1:
                    del xts[(b, g)]
                    pt = pt_pool.tile([QUAD, BLK], F32R, tag="pt")
                    nc.vector.tensor_copy(pt, ps_ps[b])
                    pts[b] = pt
                    pending.extend(back_units(b, pt))
                else:
                    del xts[(b, g)]

            pts = {}
            pending = []
            n_tot = N_BLKS * n_g
            # merged 2-group slots: [t x8][p x4][pair] -- fewer instruction-
            # type transitions on the PE stream (each t->p / p->pair boundary
            # exposes ~90-200ns of weight-load latency that chains hide).
            # tile_wait_until pins the scheduler's dispatch order to the slot
            # sequence (its internal cost model otherwise reorders the next
            # block's transposes behind a whole update phase)
            G = 0
            w = 0
            for G in range(0, n_tot + LAG, 2):
                w += 1
                with tc.tile_wait_until(w):
                    for d in range(2):
                        Gt = G + d
                        if Gt < n_tot:
                            transposes(Gt // n_g, Gt % n_g)
                    for d in range(2):
                        Gp = G + d - LAG
                        if 0 <= Gp < n_tot:
                            proj(Gp // n_g, Gp % n_g)
                    # two back pairs every other slot: back-to-back pair
                    # matmuls amortize the PE stream's entry/exit weight-load
                    # exposure (~430ns per isolated pair)
                    if (G // 2) % 2 == 0:
                        for _ in range(2):
                            if pending:
                                pending.pop(0)()
            while pending:
                w += 1
                with tc.tile_wait_until(w):
                    pending.pop(0)()

    nc.compile()
    return nc


def _get_program():
    if "nc" not in _CACHE:
        _CACHE["nc"] = _build_program()
    return _CACHE["nc"]


def kernel(input, hra_u, **run_kwargs):
    input = np.ascontiguousarray(np.asarray(input, dtype=np.float32))
    hra_u = np.asarray(hra_u, dtype=np.float32)

    A, UT = _householder_wy(hra_u)
    # pack A compactly: partition p holds A[c*128+p, :] at free offset c*R;
    # the device replicates it into 2 column groups with DVE copies
    a_packed = np.ascontiguousarray(
        A.reshape(D_CHUNKS, P, R).transpose(1, 0, 2).reshape(P, D_CHUNKS * R)
    )
    ident = np.eye(P, dtype=np.float32)

    x_flat = input.reshape(ROWS, D)
    in_maps = [
        {
            "x": x_flat[c * ROWS_PER_CORE:(c + 1) * ROWS_PER_CORE],
            "a": a_packed,
            "ut": UT,
            "ident": ident,
        }
        for c in range(N_CORES)
    ]

    nc = _get_program()
    res = run_bass_kernel_spmd(nc, in_maps, core_ids=list(range(N_CORES)),
                               **run_kwargs)
    out = np.concatenate([r["out"] for r in res.results], axis=0)
    if run_kwargs:
        kernel.last_results = res
    return out.reshape(B, S, D)


# revision 35
# speedup vs baseline: 1.0376x; 1.0082x over previous
"""HRA (Householder Reflection Adaptation) forward kernel for Trainium2.

Math: out = x @ Q with Q = prod_i (I - 2 u_i u_i^T), u_i = normalized columns
of hra_u [4096, 8].  Using the compact WY representation:
    Q = I - U T U^T      (T upper-triangular 8x8, diag=2)
    out = x - (x @ A) @ U^T,   A = U @ T
so the device only does two skinny matmuls per tile plus a subtract.

Sharding: data-parallel over rows. x [4,2048,4096] -> [8192, 4096]; each of
8 cores gets 1024 contiguous rows. A and U^T are tiny and replicated.

Per-core pipeline (256-row blocks, 4 per core, software-pipelined):

  All DMA rides ONE sync HWDGE ring: input prefetches are enqueued first, so
  the FIFO gives a clean read phase at full HBM rate (~430 GB/s), then the
  write phase streams the outputs, instead of slow mixed read/write traffic.

  front(b): per 2-chunk group: 4 PE transposes -> PSUM strip, ACT copy ->
    SBUF x^T (rounded to f32r), then the accumulating f32r proj matmul for
    the group, emitted TWO groups late so it never stalls the PE FIFO on
    the ACT copy.  A is replicated into 2 column groups on device (one
    compact DMA + 2 DVE copies), so P^T lands in PSUM already replicated at
    partitions {0-7, 32-39}.

  back(b-1), interleaved into front(b): the replicated P^T strips let the
    K=8 update matmuls run pairwise via PE row-group packing: 2 concurrent
    N=512 f32r matmuls into one 2-bank PSUM tile, one [128,1024] DVE
    subtract in place, then the out piece queues on the sync ring behind
    the remaining reads.  One pso tile per pair with bufs=2 leaves a full
    pair of slack so the PE FIFO never blocks on DVE.

  A warm-up burst on the identity (the first DMA to land) opens the PE HAM
  clock-gate before the first real transposes.  Transposes stay plain f32:
  the f32 transpose lowers to a HAM-counted matmul, while the f32r
  transpose-mode variant does not count as PE activity and lets the clock
  gate throttle the whole kernel to 1.2 GHz.
"""

import os
import sys

for _p in ("/opt/trn_rl_repo", "/root/.axon_site", "/root/.axon_site/_ro/trn_rl_repo",
           "/root/.axon_site/_ro/pypackages"):
    if os.path.isdir(_p) and _p not in sys.path:
        sys.path.append(_p)

import numpy as np

import concourse.bass as bass
import concourse.mybir as mybir
import concourse.tile as tile
from concourse import bacc
from concourse.bass_utils import run_bass_kernel_spmd

B, S, D, R = 4, 2048, 4096, 8
N_CORES = 8
ROWS = B * S                      # 8192
ROWS_PER_CORE = ROWS // N_CORES   # 1024
P = 128
D_CHUNKS = D // P                 # 32
UPD_CHUNKS = D // 512             # 8
QUAD = 40                         # replicated-proj partition span: 32 + 8

F32 = mybir.dt.float32
F32R = mybir.dt.float32r
BF16 = mybir.dt.bfloat16

_CACHE = {}


def _householder_wy(hra_u: np.ndarray):
    """Return (A, UT) f32 with out = x - (x @ A) @ UT."""
    u = hra_u.astype(np.float32)
    u = u / np.linalg.norm(u, axis=0, keepdims=True)
    U = u.astype(np.float64)
    T = np.zeros((R, R), np.float64)
    for k in range(R):
        T[k, k] = 2.0
        if k:
            T[:k, k] = -2.0 * (T[:k, :k] @ (U[:, :k].T @ U[:, k]))
    A = (U @ T).astype(np.float32)          # [D, R]
    return A, np.ascontiguousarray(u.T)     # [R, D]


J = 2                             # 128-row tiles per block
BLK = J * P                       # 256 rows per block
N_BLKS = ROWS_PER_CORE // BLK     # 4 blocks per core
LAG = 2                           # groups the proj matmul trails transposes


def _build_program():
    nc = bacc.Bacc(trn_type="TRN2")
    x = nc.dram_tensor("x", (ROWS_PER_CORE, D), F32, kind="ExternalInput")
    a = nc.dram_tensor("a", (P, D_CHUNKS * R), F32R, kind="ExternalInput")
    ut = nc.dram_tensor("ut", (R, D), F32R, kind="ExternalInput")
    ident = nc.dram_tensor("ident", (P, P), F32, kind="ExternalInput")
    out = nc.dram_tensor("out", (ROWS_PER_CORE, D), F32, kind="ExternalOutput")

    xd = x.rearrange("(b j p) d -> b p j d", p=P, j=J)
    od = out.rearrange("(b j p) d -> b p j d", p=P, j=J)

    with tile.TileContext(nc) as tc:
        with (
            tc.tile_pool(name="const", bufs=1) as const,
            tc.tile_pool(name="xp", bufs=4) as x_pool,
            tc.tile_pool(name="xtp", bufs=4) as xt_pool,
            tc.tile_pool(name="ptp", bufs=2) as pt_pool,
            tc.tile_pool(name="pst", bufs=3, space="PSUM") as pst_pool,
            tc.tile_pool(name="psp", bufs=1, space="PSUM") as psp_pool,
            tc.tile_pool(name="pso", bufs=2, space="PSUM") as pso_pool,
        ):
            # DMA order on the sync ring: ident (warm-up), compact a, block-0
            # first halves (first transposes), ut replicas, block-0 second
            # halves, blocks 1-3 (one 4MB DMA each -- fewer issues means the
            # 8 DMAHW completion lanes never starve the ring).
            ident_sb = const.tile([P, P], F32)
            nc.sync.dma_start(ident_sb, ident[:, :])
            # load a compactly (one contiguous DMA; tiny strided DMAs would
            # be descriptor-bound), then DVE builds the column replica
            a_c = const.tile([P, D_CHUNKS, R], F32R)
            nc.sync.dma_start(a_c, a.rearrange("p (c r) -> p c r", r=R))
            # x transfers stay whole per-j tiles: 16KB contiguous per
            # partition keeps HBM reads at line rate (column-half splits
            # read 8KB-strided runs and drop to ~60% bandwidth)
            xbs = []
            xb0 = x_pool.tile([P, J, D], F32, tag="xb")
            xbs.append(xb0)
            nc.sync.dma_start(xb0[:, 0], xd[0, :, 0])
            ut_sb = const.tile([QUAD, D], F32R)
            for g in range(2):
                nc.sync.dma_start(ut_sb[32 * g:32 * g + R, :], ut[:, :])
            nc.sync.dma_start(xb0[:, 1], xd[0, :, 1])
            for b in range(1, N_BLKS):
                xb = x_pool.tile([P, J, D], F32, tag="xb")
                xbs.append(xb)
                for j in range(J):
                    nc.sync.dma_start(xb[:, j], xd[b, :, j])

            a_sb = const.tile([P, D_CHUNKS, QUAD], F32R)
            for g in range(2):
                nc.vector.tensor_copy(a_sb[:, :, 32 * g:32 * g + R], a_c)

            # warm-up burst on ident only (lands first): dense f32 matmuls
            # open the PE HAM clock-gate before the first real transposes
            warm_t = pst_pool.tile([P, 2, BLK], F32, tag="ps_t")
            nc.tensor.transpose(warm_t[:, 0, :P], ident_sb, ident_sb)
            warm = pso_pool.tile([P, 2, 512], F32, tag="ps_o")
            for _ in range(40):
                nc.tensor.matmul(warm[:, 0, :P], ident_sb, ident_sb,
                                 start=True, stop=True)

            def back_units(b, pt):
                """yield per-pair units for block b's update phase: each unit
                is 2 row-group-packed N=512 f32r matmuls into one 2-bank
                PSUM tile, then a [128,1024] DVE subtract + sync-ring DMA."""
                xb = xbs[b]
                combos = [(j, c) for j in range(J) for c in range(UPD_CHUNKS)]

                def pair(s, fine=False):
                    ps_o = pso_pool.tile([P, 2, 512], F32, tag="ps_o",
                                         name="ps_o")
                    for g in range(2):
                        j, c = combos[2 * s + g]
                        nc.tensor.matmul(
                            ps_o[:, g, :],
                            pt[32 * g:32 * g + R, j * P:(j + 1) * P],
                            ut_sb[32 * g:32 * g + R, c * 512:(c + 1) * 512],
                            start=True,
                            stop=True,
                            tile_position=(32 * g, 0),
                        )
                    j, c0 = combos[2 * s]
                    if fine:
                        # last pairs: halved sub+piece granularity drains
                        # the kernel tail sooner
                        for g in range(2):
                            c = c0 + g
                            dst = xb[:, j, c * 512:(c + 1) * 512]
                            nc.vector.tensor_sub(dst, dst, ps_o[:, g, :])
                            nc.sync.dma_start(
                                od[b, :, j, c * 512:(c + 1) * 512], dst)
                    else:
                        dst = xb[:, j, c0 * 512:(c0 + 2) * 512]
                        nc.vector.tensor_sub(dst, dst, ps_o)
                        # the write queues on the sync ring behind all reads
                        nc.sync.dma_start(
                            od[b, :, j, c0 * 512:(c0 + 2) * 512], dst)

                n_pair = len(combos) // 2
                for s in range(n_pair):
                    yield lambda s=s: pair(s, b == N_BLKS - 1
                                           and s >= n_pair - 2)

            n_g = D_CHUNKS // 2
            ps_ps = {}
            xts = {}

            def transposes(b, g):
                ps_t = pst_pool.tile([P, 2, BLK], F32, tag="ps_t")
                for i in range(2):
                    k = 2 * g + i
                    for j in range(J):
                        nc.tensor.transpose(
                            ps_t[:, i, j * P:(j + 1) * P],
                            xbs[b][:, j, k * P:(k + 1) * P],
                            ident_sb,
                        )
                xt_g = xt_pool.tile([P, 2, BLK], F32R, tag="xt_g")
                nc.scalar.copy(xt_g, ps_t)
                xts[(b, g)] = xt_g

            def proj(b, g):
                if g == 0:
                    ps_ps[b] = psp_pool.tile([QUAD, BLK], F32, tag="ps_p",
                                             name="ps_p")
                for i in range(2):
                    k = 2 * g + i
                    nc.tensor.matmul(
                        ps_ps[b],
                        a_sb[:, k],
                        xts[(b, g)][:, i],
                        start=(k == 0),
                        stop=(k == D_CHUNKS - 1),
                    )
                if g == n_g - 1:
                    del xts[(b, g)]
                    pt = pt_pool.tile([QUAD, BLK], F32R, tag="pt")
                    nc.vector.tensor_copy(pt, ps_ps[b])
                    pts[b] = pt
                    pending.extend(back_units(b, pt))
                else:
                    del xts[(b, g)]

            pts = {}
            pending = []
            n_tot = N_BLKS * n_g
            # merged 2-group slots: [t x8][p x4][pair] -- fewer instruction-
            # type transitions on the PE stream (each t->p / p->pair boundary
            # exposes ~90-200ns of weight-load latency that chains hide).
            # tile_wait_until pins the scheduler's dispatch order to the slot
            # sequence (its internal cost model otherwise reorders the next
            # block's transposes behind a whole update phase)
            G = 0
            w = 0
            for G in range(0, n_tot + LAG, 2):
                w += 1
                with tc.tile_wait_until(w):
                    for d in range(2):
                        Gt = G + d
                        if Gt < n_tot:
                            transposes(Gt // n_g, Gt % n_g)
                    for d in range(2):
                        Gp = G + d - LAG
                        if 0 <= Gp < n_tot:
                            proj(Gp // n_g, Gp % n_g)
                    # two back pairs every other slot: back-to-back pair
                    # matmuls amortize the PE stream's entry/exit weight-load
                    # exposure (~430ns per isolated pair)
                    if (G // 2) % 2 == 0:
                        for _ in range(2):
                            if pending:
                                pending.pop(0)()
            while pending:
                w += 1
                with tc.tile_wait_until(w):
                    pending.pop(0)()

    nc.compile()
    return nc


def _get_program():
    if "nc" not in _CACHE:
        _CACHE["nc"] = _build_program()
    return _CACHE["nc"]


def kernel(input, hra_u, **run_kwargs):
    input = np.ascontiguousarray(np.asarray(input, dtype=np.float32))
    hra_u = np.asarray(hra_u, dtype=np.float32)

    A, UT = _householder_wy(hra_u)
    # pack A compactly: partition p holds A[c*128+p, :] at free offset c*R;
    # the device replicates it into 2 column groups with DVE copies
    a_packed = np.ascontiguousarray(
        A.reshape(D_CHUNKS, P, R).transpose(1, 0, 2).reshape(P, D_CHUNKS * R)
    )
    ident = np.eye(P, dtype=np.float32)

    x_flat = input.reshape(ROWS, D)
    in_maps = [
        {
            "x": x_flat[c * ROWS_PER_CORE:(c + 1) * ROWS_PER_CORE],
            "a": a_packed,
            "ut": UT,
            "ident": ident,
        }
        for c in range(N_CORES)
    ]

    nc = _get_program()
    res = run_bass_kernel_spmd(nc, in_maps, core_ids=list(range(N_CORES)),
                               **run_kwargs)
    out = np.concatenate([r["out"] for r in res.results], axis=0)
    if run_kwargs:
        kernel.last_results = res
    return out.reshape(B, S, D)
